# revision 4
# baseline (speedup 1.0000x reference)
"""BiLSTM + vocab projection + log_softmax Trainium2 kernel (v2).

Strategy (8 NeuronCores, batch-parallel):
  - Shard batch B=64 -> 8 rows per core. LSTM recurrence is per-batch-row,
    so each core runs the full fwd+bwd LSTM over S=128 for its 8 rows.
  - State kept transposed: H^T [32 h-part, 8 b], C^T [32 c-part, 8 b].
    Scalar gates (f,i,o) are broadcast across the 32 c-partitions by
    replicating the gate weight column 32x in the stationary matmul operand,
    so gate*state products are plain elementwise DVE ops.
  - The per-step H^T write goes directly into a transposed H table
    HtabT [65, 1024] (rows 0:32 fwd h, 32:64 bwd h, row 64 = ones for the
    output bias; col = 8*s + b). Projection lhsT tiles are direct slices.
  - Projection: logits = Hcat @ Wout + bout over V=50257, log_softmax over V.
    v2: Wout_ext [65, V] (row 64 = bout) is loaded ONCE into SBUF as fp16
    (6.4 MB resident; the load overlaps the LSTM) instead of being streamed
    from DRAM on every pass. Per 128-row tile: pass 1 computes
    sum(exp(logits)) via ACT accum_out (no max subtraction: |logits| <= ~12),
    pass 2 recomputes the matmul (fp16 PE, full rate) and writes
    logits - ln(sum) via DVE into 4096-col staging tiles DMAed to DRAM.
    Engines pipeline across row tiles: ACT exp of tile r+1 overlaps the
    store pass of tile r; output stores (206 MB/core) are the roofline.
"""

import numpy as np

V = 50257
VP = 50258                # padded even for the PE
E = 128
HS = 32
S = 128
B = 64
NCORES = 8
BL = B // NCORES          # 8 batch rows per core
ROWS = S * BL             # 1024 output rows per core
SUB = 1024                # psum tile / exp granularity (2 PSUM banks)
STG = 4096                # store granularity (4 subs per DMA)
VT = 512                  # matmul N tile (one PSUM bank)


def _ceil_div(a, b):
    return (a + b - 1) // b


def _build(nc, tile, mybir, bass, phases=("pre", "lstm", "proj")):
    from concourse.masks import make_identity

    f32 = mybir.dt.float32
    f16 = mybir.dt.float16
    AF = mybir.ActivationFunctionType
    OP = mybir.AluOpType

    # ---------------- DRAM I/O ----------------
    idx_d = nc.dram_tensor("idx", [128, 8], mybir.dt.int32, kind="ExternalInput")
    lut_d = nc.dram_tensor("lut", [V, E], f32, kind="ExternalInput")
    wx_d = nc.dram_tensor("wx", [128, 256], f32, kind="ExternalInput")
    wh_d = nc.dram_tensor("wh", [64, 128], f32, kind="ExternalInput")
    bt_d = nc.dram_tensor("bt", [64, 4], f32, kind="ExternalInput")
    ih_d = nc.dram_tensor("ih", [64, 8], f32, kind="ExternalInput")
    ic_d = nc.dram_tensor("ic", [64, 8], f32, kind="ExternalInput")
    wo_d = nc.dram_tensor("wo", [65, VP], f16, kind="ExternalInput")
    out_d = nc.dram_tensor("out", [ROWS, V], f32, kind="ExternalOutput")

    NSUB = _ceil_div(V, SUB)          # 25 subs per row tile
    NSTG = _ceil_div(V, STG)          # 13 stores per row tile

    with tile.TileContext(nc) as tc:
        with tc.tile_pool(name="persist", bufs=1) as pp:
            # persistent SBUF state
            idx_sb = pp.tile([128, 8], mybir.dt.int32)
            wh_sb = pp.tile([64, 128], f32)
            bt_sb = pp.tile([64, 4], f32)
            wx_sb = pp.tile([128, 256], f32)
            id128 = pp.tile([128, 128], f32)
            id64 = pp.tile([64, 32], f32)
            htab = pp.tile([65, 8 * S], f32)     # transposed H table (+ones row)
            ht16 = pp.tile([65, 8 * S], f16)     # fp16 copy for the projection
            cc2 = pp.tile([64, 16], f32)         # cols 0:8 C state, 8:16 cts scratch
            xt = pp.tile([128, ROWS], f32)       # X^T (E on partitions)
            xwall = pp.tile([64, 32 * S], f32)   # per-slot gate pre-activations from x
            wo_sb = pp.tile([65, VP], f16)       # resident Wout_ext (fp16)
            logz = pp.tile([128, 8], f32)        # per row-tile log-partition
            parts = [pp.tile([128, 64], f32, name=f"part{r}") for r in range(8)]

            # resident Wout load first: overlaps gather + LSTM
            nc.sync.dma_start(out=wo_sb[:], in_=wo_d[:])
            nc.sync.dma_start(out=idx_sb[:], in_=idx_d[:])
            nc.sync.dma_start(out=wh_sb[:], in_=wh_d[:])
            nc.sync.dma_start(out=bt_sb[:], in_=bt_d[:])
            nc.sync.dma_start(out=wx_sb[:], in_=wx_d[:])
            nc.gpsimd.memset(htab[64:65, :], 1.0)
            make_identity(nc, id128[:])
            make_identity(nc, id64[0:32, :])
            make_identity(nc, id64[32:64, :])
            # initial states: fwd slot 0, bwd slot 127
            nc.sync.dma_start(out=htab[0:32, 0:8], in_=ih_d[0:32, :])
            nc.sync.dma_start(out=htab[32:64, 8 * 127:8 * 128], in_=ih_d[32:64, :])
            nc.sync.dma_start(out=cc2[:, 0:8], in_=ic_d[:])

            # ---- embedding gather + X^T + XW tables + LSTM ----
            # xw chunks needed by LSTM steps 0..63 ((d0,c0) fwd slots 0..63 and
            # (d1,c1) bwd slots 64..127) are emitted first; the other two xw
            # chunks and LSTM steps 64..126 follow, so that tail pre-work
            # overlaps the early LSTM steps.
            if "pre" not in phases:
                return nc
            do_lstm = "lstm" in phases
            xw_v = xwall[:, :].rearrange("p (s g) -> p s g", g=32)
            with tc.tile_pool(name="pre", bufs=2) as gp, \
                 tc.tile_pool(name="prepsum", bufs=2, space="PSUM") as gpp, \
                 tc.tile_pool(name="lstm", bufs=3) as lp, \
                 tc.tile_pool(name="lstmpsum", bufs=2, space="PSUM") as lpp:

                def xw_block(d, c):
                    L = 32 * d
                    for g in range(4):
                        xwp = gpp.tile([64, 512], f32, tag="xwp", name="xwp")
                        nc.tensor.matmul(
                            out=xwp[L:L + 32, :],
                            lhsT=wx_sb[:, 128 * d + 32 * g:128 * d + 32 * (g + 1)],
                            rhs=xt[:, 512 * c:512 * (c + 1)],
                            start=True, stop=True,
                        )
                        nc.vector.tensor_scalar(
                            out=xw_v[L:L + 32, 64 * c:64 * (c + 1), 8 * g:8 * (g + 1)],
                            in0=xwp[L:L + 32, :].rearrange("p (s b) -> p s b", b=8),
                            scalar1=bt_sb[L:L + 32, g:g + 1],
                            scalar2=None,
                            op0=OP.add,
                        )

                def lstm_steps(t0, t1):
                    # two independent chains (fwd, bwd), interleaved; each
                    # keeps every DVE operand pair at one base partition
                    for t in range(t0, t1):
                        for d in range(2):
                            L = 32 * d
                            rs = t if d == 0 else (S - 1) - t       # read slot
                            ws = t + 1 if d == 0 else (S - 2) - t   # write slot
                            gall = lpp.tile([64, 32], f32, tag=f"gall{d}", name="gall")
                            nc.tensor.matmul(
                                out=gall[L:L + 32, :],
                                lhsT=id64[L:L + 32, :],
                                rhs=xwall[L:L + 32, 32 * rs:32 * (rs + 1)],
                                start=True, stop=False,
                            )
                            for g in range(4):
                                nc.tensor.matmul(
                                    out=gall[L:L + 32, 8 * g:8 * (g + 1)],
                                    lhsT=wh_sb[L:L + 32, 32 * g:32 * (g + 1)],
                                    rhs=htab[L:L + 32, 8 * rs:8 * (rs + 1)],
                                    start=False, stop=(g == 3),
                                    skip_group_check=(g != 3),
                                )
                            # gate cols: f 0:8, i 8:16, o 16:24, s2 24:32
                            sall = lp.tile([64, 32], f32, tag=f"sall{d}", name="sall")
                            nc.scalar.activation(
                                sall[L:L + 32, :], gall[L:L + 32, :], AF.Sigmoid)
                            # cts = 2*s2-1 next to C so [f,i]*[C,cts] is one op
                            nc.vector.tensor_scalar(
                                out=cc2[L:L + 32, 8:16], in0=sall[L:L + 32, 24:32],
                                scalar1=2.0, scalar2=-1.0, op0=OP.mult, op1=OP.add)
                            t23 = lp.tile([64, 16], f32, tag=f"t23{d}", name="t23")
                            nc.vector.tensor_tensor(
                                out=t23[L:L + 32, :], in0=sall[L:L + 32, 0:16],
                                in1=cc2[L:L + 32, 0:16], op=OP.mult)
                            nc.vector.tensor_tensor(
                                out=cc2[L:L + 32, 0:8], in0=t23[L:L + 32, 0:8],
                                in1=t23[L:L + 32, 8:16], op=OP.add)
                            th = lp.tile([64, 8], f32, tag=f"th{d}", name="th")
                            nc.scalar.activation(
                                th[L:L + 32, :], cc2[L:L + 32, 0:8], AF.Tanh)
                            nc.vector.tensor_tensor(
                                out=htab[L:L + 32, 8 * ws:8 * (ws + 1)],
                                in0=th[L:L + 32, :], in1=sall[L:L + 32, 16:24],
                                op=OP.mult)

                for r in range(8):
                    xg = gp.tile([128, 128], f32, tag="xg", name="xg")
                    nc.gpsimd.indirect_dma_start(
                        out=xg[:],
                        out_offset=None,
                        in_=lut_d[:],
                        in_offset=bass.IndirectOffsetOnAxis(
                            ap=idx_sb[:, r:r + 1], axis=0),
                    )
                    xtp = gpp.tile([128, 128], f32, tag="xtp", name="xtp")
                    nc.tensor.transpose(out=xtp[:], in_=xg[:], identity=id128[:])
                    nc.vector.tensor_copy(out=xt[:, 128 * r:128 * (r + 1)], in_=xtp[:])

                xw_block(0, 0)      # fwd slots 0..63
                xw_block(1, 1)      # bwd slots 64..127
                if do_lstm:
                    lstm_steps(0, 64)
                xw_block(0, 1)
                xw_block(1, 0)
                if do_lstm:
                    lstm_steps(64, S - 1)

            # ---------------- projection + log-softmax ----------------
            # Software-pipelined: pass-1 (matmul+exp, ACT-bound) of row tile
            # r+1 is emitted interleaved with pass-2 (matmul+subtract+store,
            # DVE/DMA-bound) of row tile r, so all engines stay busy. The
            # PSUM pool holds 4 x [128, SUB] tiles (2 banks each): the p1 and
            # p2 streams each get double buffering out of the rotation.
            if "proj" not in phases:
                return nc
            nc.vector.tensor_copy(out=ht16[:], in_=htab[:])

            def mms(jpp, lhs, k):
                s0 = k * SUB
                ss = min(SUB, V - s0)
                sse = ss + (ss % 2)
                pj = jpp.tile([128, SUB], f32, tag="pj", name="pj")
                for v in range(_ceil_div(sse, VT)):
                    vs = min(VT, sse - VT * v)
                    nc.tensor.matmul(
                        out=pj[:, VT * v:VT * v + vs],
                        lhsT=lhs,
                        rhs=wo_sb[:, s0 + VT * v:s0 + VT * v + vs],
                        start=True, stop=True,
                    )
                return pj, ss

            with tc.tile_pool(name="scrp", bufs=2) as cp, \
                 tc.tile_pool(name="stgp", bufs=3) as sp, \
                 tc.tile_pool(name="projpsum", bufs=4, space="PSUM") as jpp:

                def p1(r, k):
                    lhs = ht16[:, 128 * r:128 * (r + 1)]
                    pj, ss = mms(jpp, lhs, k)
                    scr = cp.tile([128, SUB], mybir.dt.bfloat16, tag="scr", name="scr")
                    nc.scalar.activation(
                        scr[:, :ss], pj[:, :ss], AF.Exp,
                        accum_out=parts[r][:, k:k + 1])

                def logz_calc(r):
                    ssum = cp.tile([128, 1], f32, tag="ssum", name="ssum")
                    nc.vector.tensor_reduce(
                        out=ssum[:], in_=parts[r][:, :NSUB],
                        axis=mybir.AxisListType.X, op=OP.add)
                    nc.scalar.activation(logz[:, r:r + 1], ssum[:], AF.Ln)

                def p2(r, k, stg):
                    lhs = ht16[:, 128 * r:128 * (r + 1)]
                    pj2, ss = mms(jpp, lhs, k)
                    h = k % (STG // SUB)
                    nc.vector.tensor_scalar(
                        out=stg[:, h * SUB:h * SUB + ss], in0=pj2[:, :ss],
                        scalar1=logz[:, r:r + 1], scalar2=None,
                        op0=OP.subtract)

                for k in range(NSUB):          # prologue: pass 1 of tile 0
                    p1(0, k)
                for r in range(8):
                    logz_calc(r)
                    for c in range(NSTG):
                        c0 = c * STG
                        cs = min(STG, V - c0)
                        stg = sp.tile([128, STG], f32, tag="stg", name="stg")
                        for k in range(c * (STG // SUB),
                                       min((c + 1) * (STG // SUB), NSUB)):
                            p2(r, k, stg)
                            if r + 1 < 8:
                                p1(r + 1, k)
                        nc.sync.dma_start(
                            out=out_d[128 * r:128 * (r + 1), c0:c0 + cs],
                            in_=stg[:, :cs])
    return nc


def _prep_shared(inputs):
    """Build the numpy operands shared by all cores."""
    f = lambda k: np.asarray(inputs[k], np.float32)
    Wf1, Wi1, WC1, Wo1 = f("Wf1"), f("Wi1"), f("WC1"), f("Wo1")
    Wf2, Wi2, WC2, Wo2 = f("Wf2"), f("Wi2"), f("WC2"), f("Wo2")

    def rep(w):  # [128,1] -> [128,32] replicated
        return np.tile(w, (1, 32)).astype(np.float32)

    wx = np.concatenate(
        [rep(Wf1[HS:, :]), rep(Wi1[HS:, :]), rep(Wo1[HS:, :]), 2.0 * WC1[HS:, :],
         rep(Wf2[HS:, :]), rep(Wi2[HS:, :]), rep(Wo2[HS:, :]), 2.0 * WC2[HS:, :]],
        axis=1)  # [128, 256]
    wh = np.zeros((64, 128), np.float32)
    wh[0:32] = np.concatenate(
        [rep(Wf1[:HS, :]), rep(Wi1[:HS, :]), rep(Wo1[:HS, :]), 2.0 * WC1[:HS, :]], axis=1)
    wh[32:64] = np.concatenate(
        [rep(Wf2[:HS, :]), rep(Wi2[:HS, :]), rep(Wo2[:HS, :]), 2.0 * WC2[:HS, :]], axis=1)

    bt = np.zeros((64, 4), np.float32)
    for col, (b1, b2) in enumerate(
            [("bf1", "bf2"), ("bi1", "bi2"), ("bo1", "bo2")]):
        bt[0:32, col] = f(b1)[0]
        bt[32:64, col] = f(b2)[0]
    bt[0:32, 3] = 2.0 * f("bC1")
    bt[32:64, 3] = 2.0 * f("bC2")

    ih = np.zeros((64, 8), np.float32)
    ih[0:32] = np.tile(f("Hf")[:, None], (1, 8))
    ih[32:64] = np.tile(f("Hb")[:, None], (1, 8))
    ic = np.zeros((64, 8), np.float32)
    ic[0:32] = np.tile(f("Cf")[:, None], (1, 8))
    ic[32:64] = np.tile(f("Cb")[:, None], (1, 8))

    wo = np.zeros((65, VP), np.float16)
    wo[0:64, :V] = f("Wout").astype(np.float16)
    wo[64, :V] = f("bout").astype(np.float16)

    lut = np.ascontiguousarray(f("lookup"))
    return dict(lut=lut, wx=np.ascontiguousarray(wx), wh=np.ascontiguousarray(wh),
                bt=bt, ih=ih, ic=ic, wo=wo)


def _run(inputs, trace=False):
    import concourse.bass as bass
    import concourse.mybir as mybir
    import concourse.tile as tile
    from concourse import bacc
    from concourse.bass_utils import run_bass_kernel_spmd

    nc = bacc.Bacc("TRN2", target_bir_lowering=False)
    _build(nc, tile, mybir, bass)
    nc.compile()

    shared = _prep_shared(inputs)
    ib = np.asarray(inputs["input_batch"]).astype(np.int32)  # [S, B]

    in_maps = []
    for k in range(NCORES):
        idx_flat = np.ascontiguousarray(ib[:, BL * k:BL * (k + 1)]).reshape(ROWS)
        idx_t = np.ascontiguousarray(idx_flat.reshape(8, 128).T)  # [128, 8]
        in_maps.append(dict(idx=idx_t, **shared))

    res = run_bass_kernel_spmd(nc, in_maps, core_ids=list(range(NCORES)), trace=trace)
    outs = [r["out"].reshape(S, BL, V) for r in res.results]
    return np.concatenate(outs, axis=1), res


def kernel(**inputs):
    out, _ = _run(inputs, trace=False)
    return out


if __name__ == "__main__":
    import concourse.bass as bass
    import concourse.mybir as mybir
    import concourse.tile as tile
    from concourse import bacc

    nc = bacc.Bacc("TRN2", target_bir_lowering=False)
    _build(nc, tile, mybir, bass)
    nc.compile()
    print("build ok")


# revision 5
# speedup vs baseline: 71.7351x; 71.7351x over previous
"""BiLSTM + vocab projection + log_softmax Trainium2 kernel (v2).

Strategy (8 NeuronCores, batch-parallel):
  - Shard batch B=64 -> 8 rows per core. LSTM recurrence is per-batch-row,
    so each core runs the full fwd+bwd LSTM over S=128 for its 8 rows.
  - State kept transposed: H^T [32 h-part, 8 b], C^T [32 c-part, 8 b].
    Scalar gates (f,i,o) are broadcast across the 32 c-partitions by
    replicating the gate weight column 32x in the stationary matmul operand,
    so gate*state products are plain elementwise DVE ops.
  - The per-step H^T write goes directly into a transposed H table
    HtabT [65, 1024] (rows 0:32 fwd h, 32:64 bwd h, row 64 = ones for the
    output bias; col = 8*s + b). Projection lhsT tiles are direct slices.
  - Projection: logits = Hcat @ Wout + bout over V=50257, log_softmax over V.
    v2: Wout_ext [65, V] (row 64 = bout) is loaded ONCE into SBUF as fp16
    (6.4 MB resident; the load overlaps the LSTM) instead of being streamed
    from DRAM on every pass. Per 128-row tile: pass 1 computes
    sum(exp(logits)) via ACT accum_out (no max subtraction: |logits| <= ~12),
    pass 2 recomputes the matmul (fp16 PE, full rate) and writes
    logits - ln(sum) via DVE into 4096-col staging tiles DMAed to DRAM.
    Engines pipeline across row tiles: ACT exp of tile r+1 overlaps the
    store pass of tile r; output stores (206 MB/core) are the roofline.
"""

import numpy as np

V = 50257
VP = 50258                # padded even for the PE
E = 128
HS = 32
S = 128
B = 64
NCORES = 8
BL = B // NCORES          # 8 batch rows per core
ROWS = S * BL             # 1024 output rows per core
SUB = 1024                # psum tile / exp granularity (2 PSUM banks)
STG = 4096                # store granularity (4 subs per DMA)
VT = 512                  # matmul N tile (one PSUM bank)


def _ceil_div(a, b):
    return (a + b - 1) // b


def _build(nc, tile, mybir, bass, phases=("pre", "lstm", "proj")):
    from concourse.masks import make_identity

    f32 = mybir.dt.float32
    f16 = mybir.dt.float16
    AF = mybir.ActivationFunctionType
    OP = mybir.AluOpType

    # ---------------- DRAM I/O ----------------
    idx_d = nc.dram_tensor("idx", [128, 8], mybir.dt.int32, kind="ExternalInput")
    lut_d = nc.dram_tensor("lut", [V, E], f32, kind="ExternalInput")
    wx_d = nc.dram_tensor("wx", [128, 256], f32, kind="ExternalInput")
    wh_d = nc.dram_tensor("wh", [64, 128], f32, kind="ExternalInput")
    bt_d = nc.dram_tensor("bt", [64, 4], f32, kind="ExternalInput")
    ih_d = nc.dram_tensor("ih", [64, 8], f32, kind="ExternalInput")
    ic_d = nc.dram_tensor("ic", [64, 8], f32, kind="ExternalInput")
    wo_d = nc.dram_tensor("wo", [65, VP], f16, kind="ExternalInput")
    out_d = nc.dram_tensor("out", [ROWS, V], f32, kind="ExternalOutput")

    NSUB = _ceil_div(V, SUB)          # 25 subs per row tile
    NSTG = _ceil_div(V, STG)          # 13 stores per row tile

    with tile.TileContext(nc) as tc:
        with tc.tile_pool(name="persist", bufs=1) as pp:
            # persistent SBUF state
            idx_sb = pp.tile([128, 8], mybir.dt.int32)
            wh_sb = pp.tile([64, 128], f32)
            bt_sb = pp.tile([64, 4], f32)
            wx_sb = pp.tile([128, 256], f32)
            id128 = pp.tile([128, 128], f32)
            id64 = pp.tile([64, 32], f32)
            htab = pp.tile([65, 8 * S], f32)     # transposed H table (+ones row)
            ht16 = pp.tile([65, 8 * S], f16)     # fp16 copy for the projection
            cc2 = pp.tile([64, 16], f32)         # cols 0:8 C state, 8:16 cts scratch
            xt = pp.tile([128, ROWS], f32)       # X^T (E on partitions)
            xwall = pp.tile([64, 32 * S], f32)   # per-slot gate pre-activations from x
            wo_sb = pp.tile([65, VP], f16)       # resident Wout_ext (fp16)
            logz = pp.tile([128, 8], f32)        # per row-tile log-partition
            parts = [pp.tile([128, 64], f32, name=f"part{r}") for r in range(8)]

            # resident Wout load first: overlaps gather + LSTM
            nc.sync.dma_start(out=wo_sb[:], in_=wo_d[:])
            nc.sync.dma_start(out=idx_sb[:], in_=idx_d[:])
            nc.sync.dma_start(out=wh_sb[:], in_=wh_d[:])
            nc.sync.dma_start(out=bt_sb[:], in_=bt_d[:])
            nc.sync.dma_start(out=wx_sb[:], in_=wx_d[:])
            nc.gpsimd.memset(htab[64:65, :], 1.0)
            make_identity(nc, id128[:])
            make_identity(nc, id64[0:32, :])
            make_identity(nc, id64[32:64, :])
            # initial states: fwd slot 0, bwd slot 127
            nc.sync.dma_start(out=htab[0:32, 0:8], in_=ih_d[0:32, :])
            nc.sync.dma_start(out=htab[32:64, 8 * 127:8 * 128], in_=ih_d[32:64, :])
            nc.sync.dma_start(out=cc2[:, 0:8], in_=ic_d[:])

            # ---- embedding gather + X^T + XW tables + LSTM ----
            # xw chunks needed by LSTM steps 0..63 ((d0,c0) fwd slots 0..63 and
            # (d1,c1) bwd slots 64..127) are emitted first; the other two xw
            # chunks and LSTM steps 64..126 follow, so that tail pre-work
            # overlaps the early LSTM steps.
            if "pre" not in phases:
                return nc
            do_lstm = "lstm" in phases
            xw_v = xwall[:, :].rearrange("p (s g) -> p s g", g=32)
            with tc.tile_pool(name="pre", bufs=2) as gp, \
                 tc.tile_pool(name="prepsum", bufs=2, space="PSUM") as gpp, \
                 tc.tile_pool(name="lstm", bufs=3) as lp, \
                 tc.tile_pool(name="lstmpsum", bufs=2, space="PSUM") as lpp:

                def xw_block(d, c):
                    L = 32 * d
                    for g in range(4):
                        xwp = gpp.tile([64, 512], f32, tag="xwp", name="xwp")
                        nc.tensor.matmul(
                            out=xwp[L:L + 32, :],
                            lhsT=wx_sb[:, 128 * d + 32 * g:128 * d + 32 * (g + 1)],
                            rhs=xt[:, 512 * c:512 * (c + 1)],
                            start=True, stop=True,
                        )
                        nc.vector.tensor_scalar(
                            out=xw_v[L:L + 32, 64 * c:64 * (c + 1), 8 * g:8 * (g + 1)],
                            in0=xwp[L:L + 32, :].rearrange("p (s b) -> p s b", b=8),
                            scalar1=bt_sb[L:L + 32, g:g + 1],
                            scalar2=None,
                            op0=OP.add,
                        )

                def lstm_steps(t0, t1):
                    # two independent chains (fwd, bwd), interleaved; each
                    # keeps every DVE operand pair at one base partition
                    for t in range(t0, t1):
                        for d in range(2):
                            L = 32 * d
                            rs = t if d == 0 else (S - 1) - t       # read slot
                            ws = t + 1 if d == 0 else (S - 2) - t   # write slot
                            gall = lpp.tile([64, 32], f32, tag=f"gall{d}", name="gall")
                            nc.tensor.matmul(
                                out=gall[L:L + 32, :],
                                lhsT=id64[L:L + 32, :],
                                rhs=xwall[L:L + 32, 32 * rs:32 * (rs + 1)],
                                start=True, stop=False,
                            )
                            for g in range(4):
                                nc.tensor.matmul(
                                    out=gall[L:L + 32, 8 * g:8 * (g + 1)],
                                    lhsT=wh_sb[L:L + 32, 32 * g:32 * (g + 1)],
                                    rhs=htab[L:L + 32, 8 * rs:8 * (rs + 1)],
                                    start=False, stop=(g == 3),
                                    skip_group_check=(g != 3),
                                )
                            # gate cols: f 0:8, i 8:16, o 16:24, s2 24:32
                            sall = lp.tile([64, 32], f32, tag=f"sall{d}", name="sall")
                            nc.scalar.activation(
                                sall[L:L + 32, :], gall[L:L + 32, :], AF.Sigmoid)
                            # cts = 2*s2-1 next to C so [f,i]*[C,cts] is one op
                            nc.vector.tensor_scalar(
                                out=cc2[L:L + 32, 8:16], in0=sall[L:L + 32, 24:32],
                                scalar1=2.0, scalar2=-1.0, op0=OP.mult, op1=OP.add)
                            t23 = lp.tile([64, 16], f32, tag=f"t23{d}", name="t23")
                            nc.vector.tensor_tensor(
                                out=t23[L:L + 32, :], in0=sall[L:L + 32, 0:16],
                                in1=cc2[L:L + 32, 0:16], op=OP.mult)
                            nc.vector.tensor_tensor(
                                out=cc2[L:L + 32, 0:8], in0=t23[L:L + 32, 0:8],
                                in1=t23[L:L + 32, 8:16], op=OP.add)
                            th = lp.tile([64, 8], f32, tag=f"th{d}", name="th")
                            nc.scalar.activation(
                                th[L:L + 32, :], cc2[L:L + 32, 0:8], AF.Tanh)
                            nc.vector.tensor_tensor(
                                out=htab[L:L + 32, 8 * ws:8 * (ws + 1)],
                                in0=th[L:L + 32, :], in1=sall[L:L + 32, 16:24],
                                op=OP.mult)

                for r in range(8):
                    xg = gp.tile([128, 128], f32, tag="xg", name="xg")
                    nc.gpsimd.indirect_dma_start(
                        out=xg[:],
                        out_offset=None,
                        in_=lut_d[:],
                        in_offset=bass.IndirectOffsetOnAxis(
                            ap=idx_sb[:, r:r + 1], axis=0),
                    )
                    xtp = gpp.tile([128, 128], f32, tag="xtp", name="xtp")
                    nc.tensor.transpose(out=xtp[:], in_=xg[:], identity=id128[:])
                    nc.vector.tensor_copy(out=xt[:, 128 * r:128 * (r + 1)], in_=xtp[:])

                xw_block(0, 0)      # fwd slots 0..63
                xw_block(1, 1)      # bwd slots 64..127
                if do_lstm:
                    lstm_steps(0, 64)
                xw_block(0, 1)
                xw_block(1, 0)
                if do_lstm:
                    lstm_steps(64, S - 1)

            # ---------------- projection + log-softmax ----------------
            # Software-pipelined: pass-1 (matmul+exp, ACT-bound) of row tile
            # r+1 is emitted interleaved with pass-2 (matmul+subtract+store,
            # DVE/DMA-bound) of row tile r, so all engines stay busy. The
            # PSUM pool holds 4 x [128, SUB] tiles (2 banks each): the p1 and
            # p2 streams each get double buffering out of the rotation.
            if "proj" not in phases:
                return nc
            nc.vector.tensor_copy(out=ht16[:], in_=htab[:])

            def mms(jpp, lhs, k):
                s0 = k * SUB
                ss = min(SUB, V - s0)
                sse = ss + (ss % 2)
                pj = jpp.tile([128, SUB], f32, tag="pj", name="pj")
                for v in range(_ceil_div(sse, VT)):
                    vs = min(VT, sse - VT * v)
                    nc.tensor.matmul(
                        out=pj[:, VT * v:VT * v + vs],
                        lhsT=lhs,
                        rhs=wo_sb[:, s0 + VT * v:s0 + VT * v + vs],
                        start=True, stop=True,
                    )
                return pj, ss

            # prologue: pass 1 of tile 0 with 2048-wide psum tiles (its own
            # 8-bank pool, closed before the main loop's pool opens) — fewer
            # ACT instructions on the serial ramp
            NB = _ceil_div(V, 2 * SUB)
            with tc.tile_pool(name="pro", bufs=2) as cp0, \
                 tc.tile_pool(name="propsum", bufs=2, space="PSUM") as jpp0:
                lhs0 = ht16[:, 0:128]
                for k2 in range(NB):
                    s0 = 2 * SUB * k2
                    ss = min(2 * SUB, V - s0)
                    sse = ss + (ss % 2)
                    pj = jpp0.tile([128, 2 * SUB], f32, tag="pj0", name="pj0")
                    for v in range(_ceil_div(sse, VT)):
                        vs = min(VT, sse - VT * v)
                        nc.tensor.matmul(
                            out=pj[:, VT * v:VT * v + vs],
                            lhsT=lhs0,
                            rhs=wo_sb[:, s0 + VT * v:s0 + VT * v + vs],
                            start=True, stop=True,
                        )
                    scr = cp0.tile([128, 2 * SUB], mybir.dt.bfloat16, tag="scr0", name="scr0")
                    nc.scalar.activation(
                        scr[:, :ss], pj[:, :ss], AF.Exp,
                        accum_out=parts[0][:, k2:k2 + 1])

            with tc.tile_pool(name="scrp", bufs=2) as cp, \
                 tc.tile_pool(name="stgp", bufs=3) as sp, \
                 tc.tile_pool(name="projpsum", bufs=4, space="PSUM") as jpp:

                def p1(r, k):
                    lhs = ht16[:, 128 * r:128 * (r + 1)]
                    pj, ss = mms(jpp, lhs, k)
                    scr = cp.tile([128, SUB], mybir.dt.bfloat16, tag="scr", name="scr")
                    nc.scalar.activation(
                        scr[:, :ss], pj[:, :ss], AF.Exp,
                        accum_out=parts[r][:, k:k + 1])

                def logz_calc(r, nsub):
                    ssum = cp.tile([128, 1], f32, tag="ssum", name="ssum")
                    nc.vector.tensor_reduce(
                        out=ssum[:], in_=parts[r][:, :nsub],
                        axis=mybir.AxisListType.X, op=OP.add)
                    nc.scalar.activation(logz[:, r:r + 1], ssum[:], AF.Ln)

                def p2(r, k, stg, on_act):
                    lhs = ht16[:, 128 * r:128 * (r + 1)]
                    pj2, ss = mms(jpp, lhs, k)
                    h = k % (STG // SUB)
                    if on_act:
                        nc.scalar.activation(
                            stg[:, h * SUB:h * SUB + ss], pj2[:, :ss],
                            AF.Identity, bias=nlogz[:, r:r + 1])
                    else:
                        nc.vector.tensor_scalar(
                            out=stg[:, h * SUB:h * SUB + ss], in0=pj2[:, :ss],
                            scalar1=logz[:, r:r + 1], scalar2=None,
                            op0=OP.subtract)

                nlogz = cp.tile([128, 8], f32, tag="nlz", name="nlz")
                for r in range(8):
                    logz_calc(r, NB if r == 0 else NSUB)
                    last = r == 7
                    if last:
                        nc.vector.tensor_scalar(
                            out=nlogz[:, r:r + 1], in0=logz[:, r:r + 1],
                            scalar1=-1.0, scalar2=None, op0=OP.mult)
                    for c in range(NSTG):
                        c0 = c * STG
                        cs = min(STG, V - c0)
                        stg = sp.tile([128, STG], f32, tag="stg", name="stg")
                        for k in range(c * (STG // SUB),
                                       min((c + 1) * (STG // SUB), NSUB)):
                            # in the last tile there is no p1 to interleave, so
                            # alternate the subtract onto the idle ACT engine
                            p2(r, k, stg, on_act=last and (k % 2 == 1))
                            if r + 1 < 8:
                                p1(r + 1, k)
                        nc.sync.dma_start(
                            out=out_d[128 * r:128 * (r + 1), c0:c0 + cs],
                            in_=stg[:, :cs])
    return nc


def _prep_shared(inputs):
    """Build the numpy operands shared by all cores."""
    f = lambda k: np.asarray(inputs[k], np.float32)
    Wf1, Wi1, WC1, Wo1 = f("Wf1"), f("Wi1"), f("WC1"), f("Wo1")
    Wf2, Wi2, WC2, Wo2 = f("Wf2"), f("Wi2"), f("WC2"), f("Wo2")

    def rep(w):  # [128,1] -> [128,32] replicated
        return np.tile(w, (1, 32)).astype(np.float32)

    wx = np.concatenate(
        [rep(Wf1[HS:, :]), rep(Wi1[HS:, :]), rep(Wo1[HS:, :]), 2.0 * WC1[HS:, :],
         rep(Wf2[HS:, :]), rep(Wi2[HS:, :]), rep(Wo2[HS:, :]), 2.0 * WC2[HS:, :]],
        axis=1)  # [128, 256]
    wh = np.zeros((64, 128), np.float32)
    wh[0:32] = np.concatenate(
        [rep(Wf1[:HS, :]), rep(Wi1[:HS, :]), rep(Wo1[:HS, :]), 2.0 * WC1[:HS, :]], axis=1)
    wh[32:64] = np.concatenate(
        [rep(Wf2[:HS, :]), rep(Wi2[:HS, :]), rep(Wo2[:HS, :]), 2.0 * WC2[:HS, :]], axis=1)

    bt = np.zeros((64, 4), np.float32)
    for col, (b1, b2) in enumerate(
            [("bf1", "bf2"), ("bi1", "bi2"), ("bo1", "bo2")]):
        bt[0:32, col] = f(b1)[0]
        bt[32:64, col] = f(b2)[0]
    bt[0:32, 3] = 2.0 * f("bC1")
    bt[32:64, 3] = 2.0 * f("bC2")

    ih = np.zeros((64, 8), np.float32)
    ih[0:32] = np.tile(f("Hf")[:, None], (1, 8))
    ih[32:64] = np.tile(f("Hb")[:, None], (1, 8))
    ic = np.zeros((64, 8), np.float32)
    ic[0:32] = np.tile(f("Cf")[:, None], (1, 8))
    ic[32:64] = np.tile(f("Cb")[:, None], (1, 8))

    wo = np.zeros((65, VP), np.float16)
    wo[0:64, :V] = f("Wout").astype(np.float16)
    wo[64, :V] = f("bout").astype(np.float16)

    lut = np.ascontiguousarray(f("lookup"))
    return dict(lut=lut, wx=np.ascontiguousarray(wx), wh=np.ascontiguousarray(wh),
                bt=bt, ih=ih, ic=ic, wo=wo)


def _fixup_act_tables(nc, mybir):
    """Coalesce the projection's Exp/Ln activation-table loads.

    bacc's load-insertion pass assigns Exp -> exp_and_others and Ln ->
    natural_log, which thrashes the ACT table twice per row tile (~2.6us
    per switch). Both live in natural_log_exp_and_others, so rewrite those
    loads to the combined set and drop back-to-back duplicates.
    """
    ids = {"exp_and_others": 0, "natural_log": 5, "natural_log_exp_and_others": 6}
    try:
        from concourse.hw_specs import get_activation_tables
        names = list(get_activation_tables(nc.m.arch).keys())
        ids = {n: names.index(n) for n in ids}
    except Exception:
        pass
    rewrite = {ids["exp_and_others"], ids["natural_log"]}
    combined = ids["natural_log_exp_and_others"]
    for blk in nc.main_func.blocks:
        cur = None
        keep = []
        for inst in blk.instructions:
            if isinstance(inst, mybir.InstLoadActFuncSet):
                if inst.act_func_set_id in rewrite:
                    inst.act_func_set_id = combined
                if inst.act_func_set_id == cur:
                    continue
                cur = inst.act_func_set_id
            keep.append(inst)
        if len(keep) != len(blk.instructions):
            del blk.instructions[:]
            blk.instructions.extend(keep)


def _run(inputs, trace=False):
    import concourse.bass as bass
    import concourse.mybir as mybir
    import concourse.tile as tile
    from concourse import bacc
    from concourse.bass_utils import run_bass_kernel_spmd

    nc = bacc.Bacc("TRN2", target_bir_lowering=False)
    _build(nc, tile, mybir, bass)
    nc.compile()
    _fixup_act_tables(nc, mybir)

    shared = _prep_shared(inputs)
    ib = np.asarray(inputs["input_batch"]).astype(np.int32)  # [S, B]

    in_maps = []
    for k in range(NCORES):
        idx_flat = np.ascontiguousarray(ib[:, BL * k:BL * (k + 1)]).reshape(ROWS)
        idx_t = np.ascontiguousarray(idx_flat.reshape(8, 128).T)  # [128, 8]
        in_maps.append(dict(idx=idx_t, **shared))

    res = run_bass_kernel_spmd(nc, in_maps, core_ids=list(range(NCORES)), trace=trace)
    outs = [r["out"].reshape(S, BL, V) for r in res.results]
    return np.concatenate(outs, axis=1), res


def kernel(**inputs):
    out, _ = _run(inputs, trace=False)
    return out


if __name__ == "__main__":
    import concourse.bass as bass
    import concourse.mybir as mybir
    import concourse.tile as tile
    from concourse import bacc

    nc = bacc.Bacc("TRN2", target_bir_lowering=False)
    _build(nc, tile, mybir, bass)
    nc.compile()
    print("build ok")


# revision 6
# speedup vs baseline: 83.2189x; 1.1601x over previous
"""BiLSTM + vocab projection + log_softmax Trainium2 kernel (v2).

Strategy (8 NeuronCores, batch-parallel):
  - Shard batch B=64 -> 8 rows per core. LSTM recurrence is per-batch-row,
    so each core runs the full fwd+bwd LSTM over S=128 for its 8 rows.
  - State kept transposed: H^T [32 h-part, 8 b], C^T [32 c-part, 8 b].
    Scalar gates (f,i,o) are broadcast across the 32 c-partitions by
    replicating the gate weight column 32x in the stationary matmul operand,
    so gate*state products are plain elementwise DVE ops.
  - The per-step H^T write goes directly into a transposed H table
    HtabT [65, 1024] (rows 0:32 fwd h, 32:64 bwd h, row 64 = ones for the
    output bias; col = 8*s + b). Projection lhsT tiles are direct slices.
  - Projection: logits = Hcat @ Wout + bout over V=50257, log_softmax over V.
    v2: Wout_ext [65, V] (row 64 = bout) is loaded ONCE into SBUF as fp16
    (6.4 MB resident; the load overlaps the LSTM) instead of being streamed
    from DRAM on every pass. Per 128-row tile: pass 1 computes
    sum(exp(logits)) via ACT accum_out (no max subtraction: |logits| <= ~12),
    pass 2 recomputes the matmul (fp16 PE, full rate) and writes
    logits - ln(sum) via DVE into 4096-col staging tiles DMAed to DRAM.
    Engines pipeline across row tiles: ACT exp of tile r+1 overlaps the
    store pass of tile r; output stores (206 MB/core) are the roofline.
"""

import numpy as np

V = 50257
VP = 50258                # padded even for the PE
E = 128
HS = 32
S = 128
B = 64
NCORES = 8
BL = B // NCORES          # 8 batch rows per core
ROWS = S * BL             # 1024 output rows per core
SUB = 1024                # psum tile / exp granularity (2 PSUM banks)
STG = 4096                # store granularity (4 subs per DMA)
VT = 512                  # matmul N tile (one PSUM bank)


def _ceil_div(a, b):
    return (a + b - 1) // b


def _build(nc, tile, mybir, bass, phases=("pre", "lstm", "proj")):
    from concourse.masks import make_identity

    f32 = mybir.dt.float32
    f16 = mybir.dt.float16
    AF = mybir.ActivationFunctionType
    OP = mybir.AluOpType

    # ---------------- DRAM I/O ----------------
    idx_d = nc.dram_tensor("idx", [128, 8], mybir.dt.int32, kind="ExternalInput")
    lut_d = nc.dram_tensor("lut", [V, E], f32, kind="ExternalInput")
    wx_d = nc.dram_tensor("wx", [128, 256], f32, kind="ExternalInput")
    wh_d = nc.dram_tensor("wh", [64, 128], f32, kind="ExternalInput")
    bt_d = nc.dram_tensor("bt", [64, 4], f32, kind="ExternalInput")
    ih_d = nc.dram_tensor("ih", [64, 8], f32, kind="ExternalInput")
    ic_d = nc.dram_tensor("ic", [64, 8], f32, kind="ExternalInput")
    wo_d = nc.dram_tensor("wo", [65, VP], f16, kind="ExternalInput")
    # fp16 output store (upcast to f32 host-side): output magnitudes are
    # ~2..23, fp16 ulp there is <= 0.016, far inside the 2e-2*scale gate —
    # and it halves the dominant DMA store traffic (206 -> 103 MB/core)
    out_d = nc.dram_tensor("out", [ROWS, V], f16, kind="ExternalOutput")

    NSUB = _ceil_div(V, SUB)          # 25 subs per row tile
    NSTG = _ceil_div(V, STG)          # 13 stores per row tile

    with tile.TileContext(nc) as tc:
        with tc.tile_pool(name="persist", bufs=1) as pp:
            # persistent SBUF state
            idx_sb = pp.tile([128, 8], mybir.dt.int32)
            wh_sb = pp.tile([64, 128], f32)
            bt_sb = pp.tile([64, 4], f32)
            wx_sb = pp.tile([128, 256], f32)
            id128 = pp.tile([128, 128], f32)
            id64 = pp.tile([64, 32], f32)
            htab = pp.tile([65, 8 * S], f32)     # transposed H table (+ones row)
            ht16 = pp.tile([65, 8 * S], f16)     # fp16 copy for the projection
            cc2 = pp.tile([64, 16], f32)         # cols 0:8 C state, 8:16 cts scratch
            xt = pp.tile([128, ROWS], f32)       # X^T (E on partitions)
            xwall = pp.tile([64, 32 * S], f32)   # per-slot gate pre-activations from x
            wo_sb = pp.tile([65, VP], f16)       # resident Wout_ext (fp16)
            logz = pp.tile([128, 8], f32)        # per row-tile log-partition
            parts = [pp.tile([128, 64], f32, name=f"part{r}") for r in range(8)]

            # resident Wout load first: overlaps gather + LSTM
            nc.sync.dma_start(out=wo_sb[:], in_=wo_d[:])
            nc.sync.dma_start(out=idx_sb[:], in_=idx_d[:])
            nc.sync.dma_start(out=wh_sb[:], in_=wh_d[:])
            nc.sync.dma_start(out=bt_sb[:], in_=bt_d[:])
            nc.sync.dma_start(out=wx_sb[:], in_=wx_d[:])
            nc.gpsimd.memset(htab[64:65, :], 1.0)
            make_identity(nc, id128[:])
            make_identity(nc, id64[0:32, :])
            make_identity(nc, id64[32:64, :])
            # initial states: fwd slot 0, bwd slot 127
            nc.sync.dma_start(out=htab[0:32, 0:8], in_=ih_d[0:32, :])
            nc.sync.dma_start(out=htab[32:64, 8 * 127:8 * 128], in_=ih_d[32:64, :])
            nc.sync.dma_start(out=cc2[:, 0:8], in_=ic_d[:])

            # ---- embedding gather + X^T + XW tables + LSTM ----
            # xw chunks needed by LSTM steps 0..63 ((d0,c0) fwd slots 0..63 and
            # (d1,c1) bwd slots 64..127) are emitted first; the other two xw
            # chunks and LSTM steps 64..126 follow, so that tail pre-work
            # overlaps the early LSTM steps.
            if "pre" not in phases:
                return nc
            do_lstm = "lstm" in phases
            xw_v = xwall[:, :].rearrange("p (s g) -> p s g", g=32)
            with tc.tile_pool(name="pre", bufs=2) as gp, \
                 tc.tile_pool(name="prepsum", bufs=2, space="PSUM") as gpp, \
                 tc.tile_pool(name="lstm", bufs=3) as lp, \
                 tc.tile_pool(name="lstmpsum", bufs=2, space="PSUM") as lpp:

                def xw_block(d, c):
                    L = 32 * d
                    for g in range(4):
                        xwp = gpp.tile([64, 512], f32, tag="xwp", name="xwp")
                        nc.tensor.matmul(
                            out=xwp[L:L + 32, :],
                            lhsT=wx_sb[:, 128 * d + 32 * g:128 * d + 32 * (g + 1)],
                            rhs=xt[:, 512 * c:512 * (c + 1)],
                            start=True, stop=True,
                        )
                        nc.vector.tensor_scalar(
                            out=xw_v[L:L + 32, 64 * c:64 * (c + 1), 8 * g:8 * (g + 1)],
                            in0=xwp[L:L + 32, :].rearrange("p (s b) -> p s b", b=8),
                            scalar1=bt_sb[L:L + 32, g:g + 1],
                            scalar2=None,
                            op0=OP.add,
                        )

                def lstm_steps(t0, t1):
                    # two independent chains (fwd, bwd), interleaved; each
                    # keeps every DVE operand pair at one base partition
                    for t in range(t0, t1):
                        for d in range(2):
                            L = 32 * d
                            rs = t if d == 0 else (S - 1) - t       # read slot
                            ws = t + 1 if d == 0 else (S - 2) - t   # write slot
                            gall = lpp.tile([64, 32], f32, tag=f"gall{d}", name="gall")
                            nc.tensor.matmul(
                                out=gall[L:L + 32, :],
                                lhsT=id64[L:L + 32, :],
                                rhs=xwall[L:L + 32, 32 * rs:32 * (rs + 1)],
                                start=True, stop=False,
                            )
                            for g in range(4):
                                nc.tensor.matmul(
                                    out=gall[L:L + 32, 8 * g:8 * (g + 1)],
                                    lhsT=wh_sb[L:L + 32, 32 * g:32 * (g + 1)],
                                    rhs=htab[L:L + 32, 8 * rs:8 * (rs + 1)],
                                    start=False, stop=(g == 3),
                                    skip_group_check=(g != 3),
                                )
                            # gate cols: f 0:8, i 8:16, o 16:24, s2 24:32
                            sall = lp.tile([64, 32], f32, tag=f"sall{d}", name="sall")
                            nc.scalar.activation(
                                sall[L:L + 32, :], gall[L:L + 32, :], AF.Sigmoid)
                            # cts = 2*s2-1 next to C so [f,i]*[C,cts] is one op
                            nc.vector.tensor_scalar(
                                out=cc2[L:L + 32, 8:16], in0=sall[L:L + 32, 24:32],
                                scalar1=2.0, scalar2=-1.0, op0=OP.mult, op1=OP.add)
                            t23 = lp.tile([64, 16], f32, tag=f"t23{d}", name="t23")
                            nc.vector.tensor_tensor(
                                out=t23[L:L + 32, :], in0=sall[L:L + 32, 0:16],
                                in1=cc2[L:L + 32, 0:16], op=OP.mult)
                            nc.vector.tensor_tensor(
                                out=cc2[L:L + 32, 0:8], in0=t23[L:L + 32, 0:8],
                                in1=t23[L:L + 32, 8:16], op=OP.add)
                            th = lp.tile([64, 8], f32, tag=f"th{d}", name="th")
                            nc.scalar.activation(
                                th[L:L + 32, :], cc2[L:L + 32, 0:8], AF.Tanh)
                            nc.vector.tensor_tensor(
                                out=htab[L:L + 32, 8 * ws:8 * (ws + 1)],
                                in0=th[L:L + 32, :], in1=sall[L:L + 32, 16:24],
                                op=OP.mult)

                for r in range(8):
                    xg = gp.tile([128, 128], f32, tag="xg", name="xg")
                    nc.gpsimd.indirect_dma_start(
                        out=xg[:],
                        out_offset=None,
                        in_=lut_d[:],
                        in_offset=bass.IndirectOffsetOnAxis(
                            ap=idx_sb[:, r:r + 1], axis=0),
                    )
                    xtp = gpp.tile([128, 128], f32, tag="xtp", name="xtp")
                    nc.tensor.transpose(out=xtp[:], in_=xg[:], identity=id128[:])
                    nc.vector.tensor_copy(out=xt[:, 128 * r:128 * (r + 1)], in_=xtp[:])

                xw_block(0, 0)      # fwd slots 0..63
                xw_block(1, 1)      # bwd slots 64..127
                if do_lstm:
                    lstm_steps(0, 64)
                xw_block(0, 1)
                xw_block(1, 0)
                if do_lstm:
                    lstm_steps(64, S - 1)

            # ---------------- projection + log-softmax ----------------
            # Software-pipelined: pass-1 (matmul+exp, ACT-bound) of row tile
            # r+1 is emitted interleaved with pass-2 (matmul+subtract+store,
            # DVE/DMA-bound) of row tile r, so all engines stay busy. The
            # PSUM pool holds 4 x [128, SUB] tiles (2 banks each): the p1 and
            # p2 streams each get double buffering out of the rotation.
            if "proj" not in phases:
                return nc
            nc.vector.tensor_copy(out=ht16[:], in_=htab[:])

            def mms(jpp, lhs, k):
                s0 = k * SUB
                ss = min(SUB, V - s0)
                sse = ss + (ss % 2)
                pj = jpp.tile([128, SUB], f32, tag="pj", name="pj")
                for v in range(_ceil_div(sse, VT)):
                    vs = min(VT, sse - VT * v)
                    nc.tensor.matmul(
                        out=pj[:, VT * v:VT * v + vs],
                        lhsT=lhs,
                        rhs=wo_sb[:, s0 + VT * v:s0 + VT * v + vs],
                        start=True, stop=True,
                    )
                return pj, ss

            # prologue: pass 1 of tile 0 with 2048-wide psum tiles (its own
            # 8-bank pool, closed before the main loop's pool opens) — fewer
            # ACT instructions on the serial ramp
            NB = _ceil_div(V, 2 * SUB)
            with tc.tile_pool(name="pro", bufs=2) as cp0, \
                 tc.tile_pool(name="propsum", bufs=2, space="PSUM") as jpp0:
                lhs0 = ht16[:, 0:128]
                for k2 in range(NB):
                    s0 = 2 * SUB * k2
                    ss = min(2 * SUB, V - s0)
                    sse = ss + (ss % 2)
                    pj = jpp0.tile([128, 2 * SUB], f32, tag="pj0", name="pj0")
                    for v in range(_ceil_div(sse, VT)):
                        vs = min(VT, sse - VT * v)
                        nc.tensor.matmul(
                            out=pj[:, VT * v:VT * v + vs],
                            lhsT=lhs0,
                            rhs=wo_sb[:, s0 + VT * v:s0 + VT * v + vs],
                            start=True, stop=True,
                        )
                    scr = cp0.tile([128, 2 * SUB], mybir.dt.bfloat16, tag="scr0", name="scr0")
                    nc.scalar.activation(
                        scr[:, :ss], pj[:, :ss], AF.Exp,
                        accum_out=parts[0][:, k2:k2 + 1])

            with tc.tile_pool(name="scrp", bufs=2) as cp, \
                 tc.tile_pool(name="stgp", bufs=3) as sp, \
                 tc.tile_pool(name="projpsum", bufs=4, space="PSUM") as jpp:

                def p1(r, k):
                    lhs = ht16[:, 128 * r:128 * (r + 1)]
                    pj, ss = mms(jpp, lhs, k)
                    scr = cp.tile([128, SUB], mybir.dt.bfloat16, tag="scr", name="scr")
                    nc.scalar.activation(
                        scr[:, :ss], pj[:, :ss], AF.Exp,
                        accum_out=parts[r][:, k:k + 1])

                def logz_calc(r, nsub):
                    ssum = cp.tile([128, 1], f32, tag="ssum", name="ssum")
                    nc.vector.tensor_reduce(
                        out=ssum[:], in_=parts[r][:, :nsub],
                        axis=mybir.AxisListType.X, op=OP.add)
                    nc.scalar.activation(logz[:, r:r + 1], ssum[:], AF.Ln)

                def p2(r, k, stg, on_act):
                    lhs = ht16[:, 128 * r:128 * (r + 1)]
                    pj2, ss = mms(jpp, lhs, k)
                    h = k % (STG // SUB)
                    if on_act:
                        nc.scalar.activation(
                            stg[:, h * SUB:h * SUB + ss], pj2[:, :ss],
                            AF.Identity, bias=nlogz[:, r:r + 1])
                    else:
                        nc.vector.tensor_scalar(
                            out=stg[:, h * SUB:h * SUB + ss], in0=pj2[:, :ss],
                            scalar1=logz[:, r:r + 1], scalar2=None,
                            op0=OP.subtract)

                nlogz = cp.tile([128, 8], f32, tag="nlz", name="nlz")
                for r in range(8):
                    logz_calc(r, NB if r == 0 else NSUB)
                    last = r == 7
                    nc.vector.tensor_scalar(
                        out=nlogz[:, r:r + 1], in0=logz[:, r:r + 1],
                        scalar1=-1.0, scalar2=None, op0=OP.mult)
                    for c in range(NSTG):
                        c0 = c * STG
                        cs = min(STG, V - c0)
                        stg = sp.tile([128, STG], f16, tag="stg", name="stg")
                        for k in range(c * (STG // SUB),
                                       min((c + 1) * (STG // SUB), NSUB)):
                            # with fp16 stores ACT and DVE co-limit: put every
                            # 13th subtract (and half the final tile's, which
                            # has no p1 work) on the ACT engine
                            on_act = (k % 2 == 1) if last else (k % 1000 == 999)
                            p2(r, k, stg, on_act=on_act)
                            if r + 1 < 8:
                                p1(r + 1, k)
                        nc.sync.dma_start(
                            out=out_d[128 * r:128 * (r + 1), c0:c0 + cs],
                            in_=stg[:, :cs])
    return nc


def _prep_shared(inputs):
    """Build the numpy operands shared by all cores."""
    f = lambda k: np.asarray(inputs[k], np.float32)
    Wf1, Wi1, WC1, Wo1 = f("Wf1"), f("Wi1"), f("WC1"), f("Wo1")
    Wf2, Wi2, WC2, Wo2 = f("Wf2"), f("Wi2"), f("WC2"), f("Wo2")

    def rep(w):  # [128,1] -> [128,32] replicated
        return np.tile(w, (1, 32)).astype(np.float32)

    wx = np.concatenate(
        [rep(Wf1[HS:, :]), rep(Wi1[HS:, :]), rep(Wo1[HS:, :]), 2.0 * WC1[HS:, :],
         rep(Wf2[HS:, :]), rep(Wi2[HS:, :]), rep(Wo2[HS:, :]), 2.0 * WC2[HS:, :]],
        axis=1)  # [128, 256]
    wh = np.zeros((64, 128), np.float32)
    wh[0:32] = np.concatenate(
        [rep(Wf1[:HS, :]), rep(Wi1[:HS, :]), rep(Wo1[:HS, :]), 2.0 * WC1[:HS, :]], axis=1)
    wh[32:64] = np.concatenate(
        [rep(Wf2[:HS, :]), rep(Wi2[:HS, :]), rep(Wo2[:HS, :]), 2.0 * WC2[:HS, :]], axis=1)

    bt = np.zeros((64, 4), np.float32)
    for col, (b1, b2) in enumerate(
            [("bf1", "bf2"), ("bi1", "bi2"), ("bo1", "bo2")]):
        bt[0:32, col] = f(b1)[0]
        bt[32:64, col] = f(b2)[0]
    bt[0:32, 3] = 2.0 * f("bC1")
    bt[32:64, 3] = 2.0 * f("bC2")

    ih = np.zeros((64, 8), np.float32)
    ih[0:32] = np.tile(f("Hf")[:, None], (1, 8))
    ih[32:64] = np.tile(f("Hb")[:, None], (1, 8))
    ic = np.zeros((64, 8), np.float32)
    ic[0:32] = np.tile(f("Cf")[:, None], (1, 8))
    ic[32:64] = np.tile(f("Cb")[:, None], (1, 8))

    wo = np.zeros((65, VP), np.float16)
    wo[0:64, :V] = f("Wout").astype(np.float16)
    wo[64, :V] = f("bout").astype(np.float16)

    lut = np.ascontiguousarray(f("lookup"))
    return dict(lut=lut, wx=np.ascontiguousarray(wx), wh=np.ascontiguousarray(wh),
                bt=bt, ih=ih, ic=ic, wo=wo)


def _fixup_act_tables(nc, mybir):
    """Coalesce the projection's Exp/Ln activation-table loads.

    bacc's load-insertion pass assigns Exp -> exp_and_others and Ln ->
    natural_log, which thrashes the ACT table twice per row tile (~2.6us
    per switch). Both live in natural_log_exp_and_others, so rewrite those
    loads to the combined set and drop back-to-back duplicates.
    """
    ids = {"exp_and_others": 0, "natural_log": 5, "natural_log_exp_and_others": 6}
    try:
        from concourse.hw_specs import get_activation_tables
        names = list(get_activation_tables(nc.m.arch).keys())
        ids = {n: names.index(n) for n in ids}
    except Exception:
        pass
    rewrite = {ids["exp_and_others"], ids["natural_log"]}
    combined = ids["natural_log_exp_and_others"]
    for blk in nc.main_func.blocks:
        cur = None
        keep = []
        for inst in blk.instructions:
            if isinstance(inst, mybir.InstLoadActFuncSet):
                if inst.act_func_set_id in rewrite:
                    inst.act_func_set_id = combined
                if inst.act_func_set_id == cur:
                    continue
                cur = inst.act_func_set_id
            keep.append(inst)
        if len(keep) != len(blk.instructions):
            del blk.instructions[:]
            blk.instructions.extend(keep)


def _run(inputs, trace=False):
    import concourse.bass as bass
    import concourse.mybir as mybir
    import concourse.tile as tile
    from concourse import bacc
    from concourse.bass_utils import run_bass_kernel_spmd

    nc = bacc.Bacc("TRN2", target_bir_lowering=False)
    _build(nc, tile, mybir, bass)
    nc.compile()
    _fixup_act_tables(nc, mybir)

    shared = _prep_shared(inputs)
    ib = np.asarray(inputs["input_batch"]).astype(np.int32)  # [S, B]

    in_maps = []
    for k in range(NCORES):
        idx_flat = np.ascontiguousarray(ib[:, BL * k:BL * (k + 1)]).reshape(ROWS)
        idx_t = np.ascontiguousarray(idx_flat.reshape(8, 128).T)  # [128, 8]
        in_maps.append(dict(idx=idx_t, **shared))

    res = run_bass_kernel_spmd(nc, in_maps, core_ids=list(range(NCORES)), trace=trace)
    outs = [r["out"].astype(np.float32).reshape(S, BL, V) for r in res.results]
    return np.concatenate(outs, axis=1), res


def kernel(**inputs):
    out, _ = _run(inputs, trace=False)
    return out


if __name__ == "__main__":
    import concourse.bass as bass
    import concourse.mybir as mybir
    import concourse.tile as tile
    from concourse import bacc

    nc = bacc.Bacc("TRN2", target_bir_lowering=False)
    _build(nc, tile, mybir, bass)
    nc.compile()
    print("build ok")
